# revision 9
# baseline (speedup 1.0000x reference)
"""Binarized 3x3 conv (stride 1, pad 1) + training-mode sync BatchNorm on 8 TRN2 cores.

Math: out = BN(conv2d(sign(x), sign(w)) + bias), BN over (N, H, W) per channel,
affine=False, training stats. The +bias cancels exactly inside BN (mean absorbs
it, var is shift-invariant), so it is not computed.

Distribution: data-parallel, 4 images per core. Per-channel batch statistics
are combined across cores with a tiny AllReduce of (mean, E[x^2]) so the
normalization uses exact global batch stats (sync-BN).

Device algorithm (per core):
  - binarize weights/activations to fp8e4 (+-1 exact) with the ScalarE Sign LUT
  - conv as 9 shifted matmuls per output tile with fp8 DoubleRow perf mode
    (K=256 contracted per instruction). Images live in SBUF zero-padded to
    60x60 and flattened so one contiguous 480-column moving operand covers 8
    output rows (52 of 480 columns are discarded padding garbage).
  - per-tile channel stats via VectorE bn_stats/bn_aggr on the valid columns
  - AllReduce [mean, E[x^2]] (2 KB), then (x - mean) * rsqrt(var + eps) with
    VectorE tensor_scalar, DMA out.
"""

import numpy as np

import concourse.bass as bass
import concourse.tile as tile
from concourse import bacc, bass_utils, mybir

N_CORES = 8
IMGS = 4          # images per core
CCH = 256         # channels
H = W = 56
PH = 60           # padded rows allocated (56 data + top/bottom zero + slack)
PW = 60           # padded row pitch (col 0 zero, cols 1..56 data, col 57 zero)
KK = 3
ROWS = 8          # output rows per PSUM tile
NT = H // ROWS    # 7 tiles per image
NMM = ROWS * PW   # 480 moving columns per matmul
BN_EPS = 1e-5

F32 = mybir.dt.float32
FP8 = mybir.dt.float8e4


def _emit(nc, tc, x_t, w_t, out_t, with_collective):
    x_ap = x_t.ap()      # [IMGS, 256, 56, 56]
    w_ap = w_t.ap()      # [256, 256, 3, 3]
    out_ap = out_t.ap()  # [IMGS, 256, 56, 56]

    from contextlib import ExitStack

    with ExitStack() as ctx:
        wstage = ctx.enter_context(tc.tile_pool(name="wstage", bufs=2))
        xstage = ctx.enter_context(tc.tile_pool(name="xstage", bufs=2))
        xpad_p = ctx.enter_context(tc.tile_pool(name="xpad", bufs=IMGS))
        wsb_p = ctx.enter_context(tc.tile_pool(name="wsb", bufs=1))
        osb_p = ctx.enter_context(tc.tile_pool(name="osb", bufs=2 * IMGS))
        stat_p = ctx.enter_context(tc.tile_pool(name="stats", bufs=2))
        small = ctx.enter_context(tc.tile_pool(name="small", bufs=1))
        psum_p = ctx.enter_context(tc.tile_pool(name="psum", bufs=8, space="PSUM"))
        dram = ctx.enter_context(tc.tile_pool(name="dram", bufs=2, space="DRAM"))

        # ---- weights: HBM f32 [o, i, ky, kx] -> SBUF fp8 [icp | icb, ocb, k, oc]
        wsb = wsb_p.tile([128, 2, 2, KK * KK, 128], FP8)
        for icb in range(2):
            for ocb in range(2):
                ws = wstage.tile([128, 128, KK * KK], F32)  # [icp, oc, k]
                src = w_ap[
                    ocb * 128 : (ocb + 1) * 128, icb * 128 : (icb + 1) * 128, :, :
                ].rearrange("o i ky kx -> i o (ky kx)")
                nc.sync.dma_start(out=ws[:], in_=src)
                # sign + (oc,k)->(k,oc) transpose via strided APs
                nc.scalar.sign(
                    out=wsb[:, icb, ocb, :, :],
                    in_=ws[:].rearrange("p o k -> p k o"),
                )

        # ---- activations: load f32, binarize into zero-padded fp8 images
        xpads = []
        for img in range(IMGS):
            xp = xpad_p.tile([128, 2, PH, PW], FP8)  # [icp | icb, row, col]
            nc.gpsimd.memset(
                xp[:].rearrange("p i h w -> p (i h w)").bitcast(mybir.dt.uint32), 0
            )
            xpads.append(xp)
        for img in range(IMGS):
            for icb in range(2):
                xs = xstage.tile([128, H * W], F32)
                nc.sync.dma_start(
                    out=xs[:],
                    in_=x_ap[img, icb * 128 : (icb + 1) * 128, :, :].rearrange(
                        "c h w -> c (h w)"
                    ),
                )
                nc.scalar.sign(
                    out=xpads[img][:, icb, 1 : H + 1, 1 : W + 1],
                    in_=xs[:].rearrange("p (h w) -> p h w", h=H),
                )

        # ---- conv: 9 shifted fp8 DoubleRow matmuls per [128, 480] PSUM tile
        stats = [
            stat_p.tile([128, IMGS, NT, 6], F32, name="stats") for _ in range(2)
        ]  # bn_stats per (img, row-tile), per ocb
        osbs = {}
        for ocb in range(2):
            for img in range(IMGS):
                osb = osb_p.tile([128, H * W], F32)
                osbs[(ocb, img)] = osb
                osb_v = osb[:].rearrange("p (h w) -> p h w", h=H)
                xflat = xpads[img][:].rearrange("p i h w -> p i (h w)")  # [128,2,3600]
                for t in range(NT):
                    ps = psum_p.tile([128, NMM], F32)
                    ki = 0
                    for ky in range(KK):
                        for kx in range(KK):
                            s = (ROWS * t + ky) * PW + kx
                            nc.tensor.matmul(
                                ps[:],
                                lhsT=wsb[:, :, ocb, ky * KK + kx, :],
                                rhs=xflat[:, :, s : s + NMM],
                                start=(ki == 0),
                                stop=(ki == 8),
                                perf_mode=mybir.MatmulPerfMode.DoubleRow,
                            )
                            ki += 1
                    psv = ps[:].rearrange("p (r w) -> p r w", r=ROWS)[:, :, 0:W]
                    nc.scalar.copy(out=osb_v[:, t * ROWS : (t + 1) * ROWS, :], in_=psv)
                    nc.vector.bn_stats(
                        out=stats[ocb][:, img, t, :],
                        in_=osb[:, t * ROWS * W : (t + 1) * ROWS * W],
                    )

        # ---- local stats -> (mean, E[x^2]) -> AllReduce -> rstd
        mv = small.tile([128, 2, 2], F32)
        for ocb in range(2):
            nc.vector.bn_aggr(
                out=mv[:, ocb, :],
                in_=stats[ocb][:].rearrange("p n t s -> p (n t s)"),
            )
        send = small.tile([128, 4], F32)
        for ocb in range(2):
            nc.vector.tensor_copy(out=send[:, ocb : ocb + 1], in_=mv[:, ocb, 0:1])
            # q = var + mean^2  (= local E[x^2])
            nc.vector.tensor_scalar(
                out=send[:, 2 + ocb : 3 + ocb],
                in0=mv[:, ocb, 0:1],
                scalar1=mv[:, ocb, 0:1],
                scalar2=mv[:, ocb, 1:2],
                op0=mybir.AluOpType.mult,
                op1=mybir.AluOpType.add,
            )
        if with_collective:
            cin = dram.tile([128, 4], F32)
            cout = dram.tile([128, 4], F32)
            nc.gpsimd.dma_start(out=cin[:], in_=send[:])
            nc.gpsimd.collective_compute(
                "AllReduce",
                mybir.AluOpType.add,
                replica_groups=[list(range(N_CORES))],
                ins=[cin.opt()],
                outs=[cout.opt()],
            )
            recv = small.tile([128, 4], F32)
            nc.gpsimd.dma_start(out=recv[:], in_=cout[:])
            inv_n = 1.0 / N_CORES
        else:
            recv = send
            inv_n = 1.0

        meang = small.tile([128, 2], F32)
        varg = small.tile([128, 2], F32)
        rstd = small.tile([128, 2], F32)
        eps_t = small.tile([128, 1], F32)
        nc.vector.memset(eps_t[:], BN_EPS)
        for ocb in range(2):
            nc.vector.tensor_scalar(
                out=meang[:, ocb : ocb + 1],
                in0=recv[:, ocb : ocb + 1],
                scalar1=inv_n,
                scalar2=None,
                op0=mybir.AluOpType.mult,
            )
            # var = E[x^2] - mean^2
            nc.vector.tensor_scalar(
                out=varg[:, ocb : ocb + 1],
                in0=meang[:, ocb : ocb + 1],
                scalar1=meang[:, ocb : ocb + 1],
                scalar2=None,
                op0=mybir.AluOpType.mult,
            )
            nc.vector.tensor_scalar(
                out=varg[:, ocb : ocb + 1],
                in0=recv[:, 2 + ocb : 3 + ocb],
                scalar1=inv_n,
                scalar2=varg[:, ocb : ocb + 1],
                op0=mybir.AluOpType.mult,
                op1=mybir.AluOpType.subtract,
            )
            # rstd = 1 / sqrt(var + eps)
            nc.scalar.activation(
                out=rstd[:, ocb : ocb + 1],
                in_=varg[:, ocb : ocb + 1],
                func=mybir.ActivationFunctionType.Sqrt,
                bias=eps_t[:],
            )
            nc.vector.reciprocal(
                out=rstd[:, ocb : ocb + 1], in_=rstd[:, ocb : ocb + 1]
            )

        # ---- normalize + store
        for ocb in range(2):
            for img in range(IMGS):
                osb = osbs[(ocb, img)]
                nc.vector.tensor_scalar(
                    out=osb[:],
                    in0=osb[:],
                    scalar1=meang[:, ocb : ocb + 1],
                    scalar2=rstd[:, ocb : ocb + 1],
                    op0=mybir.AluOpType.subtract,
                    op1=mybir.AluOpType.mult,
                )
                nc.sync.dma_start(
                    out=out_ap[img, ocb * 128 : (ocb + 1) * 128, :, :].rearrange(
                        "c h w -> c (h w)"
                    ),
                    in_=osb[:],
                )


def build_nc(with_collective=True, num_devices=N_CORES):
    nc = bacc.Bacc(
        "TRN2", target_bir_lowering=False, debug=False, num_devices=num_devices
    )
    x_t = nc.dram_tensor("x", [IMGS, CCH, H, W], F32, kind="ExternalInput")
    w_t = nc.dram_tensor("w", [CCH, CCH, KK, KK], F32, kind="ExternalInput")
    out_t = nc.dram_tensor("out", [IMGS, CCH, H, W], F32, kind="ExternalOutput")
    with tile.TileContext(nc) as tc:
        _emit(nc, tc, x_t, w_t, out_t, with_collective)
    nc.compile()
    return nc


_NC_CACHE = {}


def _get_nc():
    if "nc" not in _NC_CACHE:
        _NC_CACHE["nc"] = build_nc()
    return _NC_CACHE["nc"]


def kernel(**inputs) -> np.ndarray:
    x = np.ascontiguousarray(np.asarray(inputs["x"], dtype=np.float32))
    w = np.ascontiguousarray(np.asarray(inputs["weight"], dtype=np.float32))
    # bias is mathematically irrelevant: BN(out + b) == BN(out) for per-channel
    # bias under training-mode BN with affine=False.
    nc = _get_nc()
    in_maps = [
        {"x": np.ascontiguousarray(x[c * IMGS : (c + 1) * IMGS]), "w": w}
        for c in range(N_CORES)
    ]
    res = bass_utils.run_bass_kernel_spmd(
        nc, in_maps, core_ids=list(range(N_CORES)), trace=False
    )
    return np.concatenate(
        [res.results[c]["out"] for c in range(N_CORES)], axis=0
    ).astype(np.float32)
